# revision 38
# baseline (speedup 1.0000x reference)
"""Trainium2 Bass kernel for nn_ExpandEvecs.

Computes, for evecs [B=4, C=1, N=1024, K=16]:
    outers[b,k,i,j] = evecs[b,0,i,k] * evecs[b,0,j,k]
    cube = cumsum(outers, axis=k)  ->  [B, K, N, N]
i.e. cube[b,l] = V[:, :l+1] @ V[:, :l+1]^T  (Gram expansion per level).

This is an HBM-write-bound problem (the full f32 output is 256 MiB
against a 256 KiB input; HBM-per-NeuronCore is ~358 GB/s). The kernel
therefore minimizes device-side output bytes; the host only *moves*
data afterwards (dtype upconvert, strided scatter, symmetric mirror) —
all arithmetic happens on device.

 1. Every level matrix V V^T is symmetric: only the 36 upper-triangular
    128x128 blocks of each 8x8 block grid are computed and stored
    (56.25% of elements). The host mirrors the 28 strictly-upper blocks
    into the lower triangle.
 2. Outputs are stored as fp16 (2 bytes): quantization error ~2^-11
    relative, and the end-to-end rel error vs the f32 reference is
    ~3.5e-3 against the 2e-2 gate (inputs are bf16 on the PE).

Per-core bytes drop 32 MiB -> 9.4 MiB, i.e. a ~26 us DMA roofline.

Sharding (SPMD: one program, per-core differences in DATA only):
core c = 2b + g handles batch b and levels 8g..8g+7 (slot s = level
8g+s). All matmuls use contraction depth 16; the lhsT for slot s is a
host-prepared copy of bf16(V^T) with rows > level zero-masked, so the
accumulated Gram is truncated at the right rank while every core runs
identical shapes. Contraction depth is free on the PE (cost is
free-dim-bound), so the masking costs nothing.

Per slot: the 36 upper blocks form a 4608-col concat stream (strip i =
rows 128i..128(i+1), cols 128i..1024), cut into <=512-col matmul chunks
at strip/PSUM-bank boundaries. Chunks alternate between PE row groups
at partitions 0/64 (tile_position row tiling: contraction 16 <= 32, so
two matmuls stream concurrently; group = PSUM-bank parity so concurrent
matmuls never share a bank). Five [128,1024] PSUM tiles per slot are
drained by one f32->fp16 cast copy each (PSUM source caps DVE/ACT at
1 elem/cycle; ACT at 1.2 GHz takes tiles 0/2/4, DVE at 0.96 GHz takes
1/3) into a [128,4608] staging tile, stored with 2 DMAs per slot
(finer on the first/last slot for ramp/tail). Steady state saturates
the 16 SDMA engines (~24 GB/s each); measured ~42 us vs the ~9 us
fixed boot/teardown + 26 us write roofline.
"""

import numpy as np
import ml_dtypes

import concourse.mybir as mybir
from concourse import bacc, bass
from concourse.tile import TileContext
from concourse.bass_utils import run_bass_kernel_spmd

B, C, N, K = 4, 1, 1024, 16
NCORES = 8
NB = N // 128            # 8 block-rows
SLOTS = K // 2           # levels per core
STRIP_F = [(NB - i) * 128 for i in range(NB)]          # strip free sizes
OFF = [sum(STRIP_F[:i]) for i in range(NB)]            # concat offsets
TOT = sum(STRIP_F)                                     # 4608 cols per level

# Matmul chunks: cut the 4608-col concat stream at strip boundaries
# (lhsT changes), 1024-col PSUM-tile boundaries (drain granularity) and
# 512-col PSUM-bank boundaries within each tile (a matmul must not
# cross a bank). (strip, concat col, width) triples:
CHUNKS = []
for i in range(NB):
    o = OFF[i]
    end = OFF[i] + STRIP_F[i]
    while o < end:
        nxt = min(end, (o // 512 + 1) * 512)
        CHUNKS.append((i, o, nxt - o))
        o = nxt
# PSUM tiles: [1024k, 1024(k+1)) col ranges of the concat stream
PTILES = [(k * 1024, min(TOT, (k + 1) * 1024)) for k in range((TOT + 1023) // 1024)]

F32 = mybir.dt.float32
F16 = mybir.dt.float16
BF16 = mybir.dt.bfloat16
BF16_NP = ml_dtypes.bfloat16

_nc_cache = None


def _build():
    nc = bacc.Bacc(None, target_bir_lowering=False)
    # hd: rhs (cols 0:N) + slot-0 masked weights (cols N:2N) combined so
    # one DMA per PE row group unblocks the first matmuls (each DMA's
    # ~2us completion receipt is paid once, not twice)
    hd_d = nc.declare_dram_parameter("hd", [K, 2 * N], BF16, isOutput=False)
    # slot-s zero-masked weights at cols [s*N, (s+1)*N) (free-dim packing:
    # matmul lhsT requires base partition 0)
    tl_d = nc.declare_dram_parameter("tl", [K, 8 * N], BF16, isOutput=False)
    out_d = nc.declare_dram_parameter("out", [SLOTS, 128, TOT], F16, isOutput=True)

    with TileContext(nc) as tc:
        with (
            tc.tile_pool(name="vpool", bufs=1) as vpool,
            tc.tile_pool(name="stage", bufs=8) as stage,
            tc.tile_pool(name="psum", bufs=4, space=bass.MemorySpace.PSUM) as psum,
        ):
            # weights/rhs duplicated at partitions 0:16 and 64:80 -> two
            # PE row groups (tile_position (0,0) / (64,0)) stream matmuls
            # concurrently (contraction 16 <= 32), ~2x PE throughput.
            hd = vpool.tile([128, 2 * N], BF16)  # rhs + slot-0 weights
            tl = vpool.tile([128, 8 * N], BF16)
            for g, eng in ((0, nc.scalar), (64, nc.sync)):
                eng.dma_start(out=hd[g:g + K, :], in_=hd_d[:])
                eng.dma_start(out=tl[g:g + K, :], in_=tl_d[:])
            t = hd  # rhs = hd[:, :N]

            for s in range(SLOTS):
                lhs = hd if s == 0 else tl
                c0 = N if s == 0 else s * N
                st = stage.tile([128, TOT], F16, tag="st")
                for k, (lo, hi) in enumerate(PTILES):
                    ps = psum.tile([128, 1024], F32, tag="ps")
                    for (i, o, w) in CHUNKS:
                        if not (lo <= o < hi):
                            continue
                        ro = 128 * i + (o - OFF[i])  # rhs col
                        # PE row group = PSUM bank parity: concurrent
                        # cross-group matmuls never share a PSUM bank
                        g = 64 * ((o // 512) % 2)
                        nc.tensor.matmul(
                            ps[:, o - lo:o - lo + w],
                            lhsT=lhs[g:g + K, c0 + 128 * i:c0 + 128 * (i + 1)],
                            rhs=t[g:g + K, ro:ro + w],
                            start=True,
                            stop=True,
                        )
                    # one f32->fp16 cast copy per PSUM tile (PSUM source
                    # caps DVE/ACT at 1 elem/cycle; amortize the per-op
                    # overhead). ACT (1.2 GHz) is faster than DVE
                    # (0.96 GHz): ACT drains tiles 0/2/4, DVE 1/3.
                    # Slot 0's first tile is drained in halves by both
                    # engines so the first store fires earlier.
                    if s == 0 and k == 0:
                        nc.scalar.copy(st[:, :512], ps[:, :512])
                        nc.sync.dma_start(out=out_d[0, :, :512], in_=st[:, :512])
                        nc.vector.tensor_copy(st[:, 512:1024], ps[:, 512:1024])
                    elif k in (0, 2, 4):
                        nc.scalar.copy(st[:, lo:hi], ps[:, :hi - lo])
                    else:
                        nc.vector.tensor_copy(st[:, lo:hi], ps[:, :hi - lo])
                    # HWDGE descriptor-gen on SP costs ~775ns per store
                    # regardless of size (128 descriptors) -> few big
                    # stores; slot 0 (ramp) and slot 7 (tail) are split
                    # finer so the DMA stream starts earlier / drains
                    # the last bytes sooner.
                    cuts = {
                        0: {0: 512, 1: 1024, 3: 2048, 4: 4096},
                        SLOTS - 1: {1: 0, 3: 2048, 4: 4096},
                    }.get(s, {1: 0, 3: 2048, 4: 4096})
                    if k in cuts:
                        slo = cuts[k]
                        nc.sync.dma_start(
                            out=out_d[s, :, slo:hi], in_=st[:, slo:hi]
                        )

    nc.compile()
    return nc


def _get_nc():
    global _nc_cache
    if _nc_cache is None:
        _nc_cache = _build()
    return _nc_cache


def _prepare_in_maps(evecs: np.ndarray) -> list[dict]:
    in_maps = []
    for c in range(NCORES):
        b, g = divmod(c, 2)
        vt = np.ascontiguousarray(evecs[b, 0].T, dtype=np.float32)  # [K, N]
        a = vt.astype(BF16_NP)
        tl = np.zeros((K, 8 * N), dtype=BF16_NP)
        for s in range(SLOTS):
            lvl = 8 * g + s
            tl[:lvl + 1, s * N:(s + 1) * N] = a[:lvl + 1]
        hd = np.zeros((K, 2 * N), dtype=BF16_NP)
        hd[:, :N] = a
        hd[:, N:] = tl[:, :N]  # slot-0 masked weights
        in_maps.append({"hd": hd, "tl": tl})
    return in_maps


def _assemble(results: list[dict]) -> np.ndarray:
    out = np.empty((B, K, N, N), dtype=np.float32)
    for c in range(NCORES):
        b, g = divmod(c, 2)
        buf = np.asarray(results[c]["out"]).astype(np.float32)  # [8,128,4608]
        lv = slice(8 * g, 8 * g + SLOTS)
        for i in range(NB):
            out[b, lv, 128 * i:128 * (i + 1), 128 * i:] = \
                buf[:, :, OFF[i]:OFF[i] + STRIP_F[i]]
    # mirror the strictly-upper 128x128 blocks into the lower triangle
    for i in range(NB):
        for j in range(i + 1, NB):
            out[:, :, 128 * j:128 * (j + 1), 128 * i:128 * (i + 1)] = \
                out[:, :, 128 * i:128 * (i + 1), 128 * j:128 * (j + 1)] \
                .transpose(0, 1, 3, 2)
    return out.reshape(B, K * C, N, N)


def kernel(evecs) -> np.ndarray:
    evecs = np.asarray(evecs, dtype=np.float32)
    assert evecs.shape == (B, C, N, K), evecs.shape
    nc = _get_nc()
    in_maps = _prepare_in_maps(evecs)
    last_err = None
    for _attempt in range(3):
        try:
            r = run_bass_kernel_spmd(nc, in_maps, list(range(NCORES)))
            return _assemble(r.results)
        except Exception as e:  # transient NRT/device hiccups: retry
            last_err = e
    raise last_err
